# revision 7
# baseline (speedup 1.0000x reference)
"""Trainium2 kernel for nn_ConvNN_2D_Spatial_K_N_Location.

Strategy (8 NeuronCores):
  - The two KNN-conv layers (irregular top-9 selection, ~6% of FLOPs) run on
    host in fp32 with reference tie-breaking, using a candidate-projection
    table so per-token work is a 9-row cache-resident gather. The
    shuffle->unshuffle round trip between the layers cancels and is skipped.
  - The dominant fc1 (1024x32768x1024) is contraction-sharded across the 8
    cores: core j gets 1/8 of h2^T and 1/8 of fw1^T (bf16), so nothing is
    replicated over the slow host link. Partials are summed with an on-device
    ReduceScatter that also hands each core its 128-row batch shard for fc2.
  - Everything is pipelined: fw1 is device_put asynchronously up front, the
    batch is processed in 4 quarters (conv on host overlapping h2-quarter
    transfers and device calls), and the bass/XLA compile runs in a side
    thread while the first conv quarters compute.
"""
import threading
import numpy as np
import ml_dtypes

import jax
from jax.sharding import Mesh, NamedSharding, PartitionSpec
from jax.experimental.shard_map import shard_map

import concourse.bass as bass
import concourse.tile as tile
from concourse import bacc, mybir
from concourse import bass2jax as B2J

K, N, SCALE = 9, 8, 2
NCORES = 8
B = 1024
BQ = 256               # batch rows per device call (4 calls)
BL = BQ // NCORES      # 32 out rows per core per call
F = 32768              # fc1 contraction
FSH = F // NCORES      # 4096 per core
U = 1024               # fc1 output
O2 = 10                # final outputs

_CACHE = {}
BF16 = ml_dtypes.bfloat16


# ---------------------------------------------------------------- host conv
def _unshuffle(x, s):
    b, c, h, w = x.shape
    return x.reshape(b, c, h//s, s, w//s, s).transpose(0, 1, 3, 5, 2, 4).reshape(b, c*s*s, h//s, w//s)


def _shuffle(x, s):
    b, c, h, w = x.shape
    return x.reshape(b, c//(s*s), s, s, h, w).transpose(0, 1, 4, 2, 5, 3).reshape(b, c//(s*s), h*s, w*s)


def _conv_core(xc, w, bvec, H, W):
    """KNN conv on channel-major tokens. xc: (nb, C, H*W) -> (nb, Cout, H*W)."""
    nb, C, HW = xc.shape
    Cf = C + 2
    Cout = w.shape[0]
    xf = np.empty((nb, Cf, HW), np.float32)
    xf[:, :C] = xc
    gy, gx = np.meshgrid(np.linspace(0., 1., H, dtype=np.float32),
                         np.linspace(0., 1., W, dtype=np.float32), indexing='ij')
    xf[:, C] = gy.ravel()
    xf[:, C+1] = gx.ravel()
    ih = np.linspace(0, H-1, N).astype(np.int32)
    iw = np.linspace(0, W-1, N).astype(np.int32)
    cols = (ih[:, None] * W + iw[None, :]).ravel()
    samp = np.ascontiguousarray(xf[:, :, cols])            # (nb, Cf, 64)
    xt = np.ascontiguousarray(xf.transpose(0, 2, 1))       # (nb, HW, Cf)
    d2 = np.matmul(xt, samp)
    d2 *= -2.0
    d2 += np.einsum('bct,bct->bt', xf, xf)[:, :, None]
    d2 += np.einsum('bcn,bcn->bn', samp, samp)[:, None, :]
    # top-K nearest, ties toward lower candidate index (== jax top_k)
    idx = np.argsort(d2, axis=2, kind='stable')[:, :, :K]  # (nb, HW, K)
    # candidate projection table: Ptab[b, n, k, o] = sum_f samp[b,f,n] w[o,f,k]
    w2d = np.ascontiguousarray(w.transpose(1, 2, 0).reshape(Cf, K * Cout))
    st = np.ascontiguousarray(samp.transpose(0, 2, 1))     # (nb, 64, Cf)
    Ptab = np.matmul(st, w2d).reshape(nb, N*N, K, Cout)
    # chunk over images so the accumulator stays cache-resident
    out = np.empty((nb, HW, Cout), np.float32)
    CH = 16
    for c0 in range(0, nb, CH):
        sl = slice(c0, c0 + CH)
        Pc = Ptab[sl]
        ic = idx[sl]
        bi = np.arange(Pc.shape[0])[:, None]
        acc = Pc[bi, ic[:, :, 0], 0]                       # (CH, HW, Cout)
        for k in range(1, K):
            acc += Pc[bi, ic[:, :, k], k]
        out[sl] = acc
    out += bvec
    return np.ascontiguousarray(out.transpose(0, 2, 1))    # (nb, Cout, HW)


def _host_convs_q(xq, w1, b1, w2, b2):
    nb = xq.shape[0]
    a1 = _unshuffle(xq, SCALE).reshape(nb, 12, 256)
    o1 = _conv_core(a1, w1, b1, 16, 16)                    # (nb, 64, 256)
    np.maximum(o1, 0., out=o1)
    # shuffle -> unshuffle between the layers cancels exactly
    o2 = _conv_core(o1, w2, b2, 16, 16)                    # (nb, 128, 256)
    np.maximum(o2, 0., out=o2)
    return _shuffle(o2.reshape(nb, 128, 16, 16), SCALE).reshape(nb, F)


# ---------------------------------------------------------------- device fc
def _build_fc_kernel():
    nc = bacc.Bacc("TRN2", target_bir_lowering=False, debug=False,
                   enable_asserts=False, num_devices=NCORES)
    f32 = mybir.dt.float32
    bf16 = mybir.dt.bfloat16
    h2t = nc.dram_tensor("h2t", (FSH, BQ), bf16, kind="ExternalInput").ap()
    fw1t = nc.dram_tensor("fw1t", (FSH, U), bf16, kind="ExternalInput").ap()
    fb1r = nc.dram_tensor("fb1r", (1, U), bf16, kind="ExternalInput").ap()
    fw2t = nc.dram_tensor("fw2t", (U, O2), f32, kind="ExternalInput").ap()
    fb2r = nc.dram_tensor("fb2r", (1, O2), f32, kind="ExternalInput").ap()
    onesr = nc.dram_tensor("onesr", (1, BL), f32, kind="ExternalInput").ap()
    onesb = nc.dram_tensor("onesb", (1, 128), bf16, kind="ExternalInput").ap()
    ident = nc.dram_tensor("ident", (128, 128), f32, kind="ExternalInput").ap()
    outt = nc.dram_tensor("outt", (O2, BL), f32, kind="ExternalOutput").ap()

    NB = BQ // 128       # 2 batch blocks per call
    NF = FSH // 128      # 32 contraction tiles per core
    with tile.TileContext(nc) as tc:
        with tc.tile_pool(name="w", bufs=NF) as wpool, \
             tc.tile_pool(name="h", bufs=2 * NF) as hpool, \
             tc.tile_pool(name="small", bufs=1) as spool, \
             tc.tile_pool(name="acts", bufs=4) as apool, \
             tc.tile_pool(name="dram", bufs=1, space="DRAM") as dpool, \
             tc.tile_pool(name="ps", bufs=4, space="PSUM") as pspool, \
             tc.tile_pool(name="pst", bufs=2, space="PSUM") as ptpool:

            ones_t = spool.tile([1, BL], f32)
            nc.sync.dma_start(ones_t[:], onesr[:, :])
            onesb_t = spool.tile([1, 128], bf16)
            nc.sync.dma_start(onesb_t[:], onesb[:, :])
            fb1_t = spool.tile([1, U], bf16)
            nc.sync.dma_start(fb1_t[:], fb1r[:, :])
            fb2_t = spool.tile([1, O2], f32)
            nc.sync.dma_start(fb2_t[:], fb2r[:, :])
            id_t = spool.tile([128, 128], f32)
            nc.sync.dma_start(id_t[:], ident[:, :])
            fw2_t = spool.tile([128, 8 * O2], f32)
            for c in range(8):
                nc.sync.dma_start(fw2_t[:, bass.ts(c, O2)],
                                  fw2t[bass.ts(c, 128), :])

            # fw1 shard fully resident in SBUF (8 MB bf16)
            wt = []
            for i in range(NF):
                t = wpool.tile([128, U], bf16)
                nc.sync.dma_start(t[:], fw1t[bass.ts(i, 128), :])
                wt.append(t)

            cc_in = dpool.tile([BQ, U], f32)
            cc_out = dpool.tile([BL, U], f32)

            for bb in range(NB):
                ht = []
                for i in range(NF):
                    t = hpool.tile([128, 128], bf16)
                    nc.sync.dma_start(t[:], h2t[bass.ts(i, 128), bass.ts(bb, 128)])
                    ht.append(t)
                for uh in range(2):
                    ps = pspool.tile([128, 512], f32)
                    for i in range(NF):
                        nc.tensor.matmul(ps[:], lhsT=ht[i][:],
                                         rhs=wt[i][:, bass.ts(uh, 512)],
                                         start=(i == 0), stop=False)
                    # bias (only core 0's fb1r is nonzero)
                    nc.tensor.matmul(ps[:], lhsT=onesb_t[:],
                                     rhs=fb1_t[:, bass.ts(uh, 512)],
                                     start=False, stop=True)
                    pa = apool.tile([128, 512], f32)
                    nc.scalar.copy(pa[:], ps[:])
                    nc.sync.dma_start(cc_in[bass.ts(bb, 128), bass.ts(uh, 512)],
                                      pa[:])

            nc.gpsimd.collective_compute(
                "ReduceScatter", mybir.AluOpType.add,
                replica_groups=[list(range(NCORES))],
                ins=[cc_in.opt()], outs=[cc_out.opt()])

            h1 = apool.tile([BL, U], f32)
            nc.sync.dma_start(h1[:], cc_out[:])
            h1r = apool.tile([BL, U], f32)
            nc.scalar.activation(h1r[:], h1[:],
                                 mybir.ActivationFunctionType.Relu)

            # transpose h1r in (BL x 128) blocks (PE), then fc2
            h1T = apool.tile([128, 8 * BL], f32)
            for c in range(8):
                pt = ptpool.tile([128, BL], f32)
                nc.tensor.transpose(pt[:], h1r[:, bass.ts(c, 128)],
                                    id_t[0:BL, 0:BL])
                nc.scalar.copy(h1T[:, bass.ts(c, BL)], pt[:])

            psum2 = ptpool.tile([O2, BL], f32)
            for c in range(8):
                nc.tensor.matmul(psum2[:], lhsT=fw2_t[:, bass.ts(c, O2)],
                                 rhs=h1T[:, bass.ts(c, BL)],
                                 start=(c == 0), stop=False)
            nc.tensor.matmul(psum2[:], lhsT=fb2_t[:], rhs=ones_t[:],
                             start=False, stop=True)

            out_t = apool.tile([O2, BL], f32)
            nc.scalar.copy(out_t[:], psum2[:])
            nc.sync.dma_start(outt[:, :], out_t[:])

    nc.compile()
    return nc


def _jit_compile(nc, mesh):
    """Build + AOT-compile the sharded executable (run_bass_via_pjrt's path,
    without its host-side concat: we pass pre-sharded device arrays)."""
    B2J.install_neuronx_cc_hook()
    partition_name = nc.partition_id_tensor.name if nc.partition_id_tensor else None
    in_names, out_names, out_avals, zero_shapes = [], [], [], []
    for alloc in nc.m.functions[0].allocations:
        if not isinstance(alloc, mybir.MemoryLocationSet):
            continue
        name = alloc.memorylocations[0].name
        if alloc.kind == "ExternalInput":
            if name != partition_name:
                in_names.append(name)
        elif alloc.kind == "ExternalOutput":
            shape = tuple(alloc.tensor_shape)
            dtype = mybir.dt.np(alloc.dtype)
            out_names.append(name)
            out_avals.append(jax.core.ShapedArray(shape, dtype))
            zero_shapes.append((shape, dtype))
    n_params = len(in_names)
    n_outs = len(out_names)
    all_names = list(in_names) + list(out_names)
    if partition_name is not None:
        all_names.append(partition_name)

    def _body(*args):
        operands = list(args)
        if partition_name is not None:
            operands.append(B2J.partition_id_tensor())
        outs = B2J._bass_exec_p.bind(
            *operands,
            out_avals=tuple(out_avals),
            in_names=tuple(all_names),
            out_names=tuple(out_names),
            lowering_input_output_aliases=(),
            sim_require_finite=True,
            sim_require_nnan=True,
            nc=nc,
        )
        return tuple(outs)

    donate = tuple(range(n_params, n_params + n_outs))
    in_specs = (PartitionSpec("core"),) * (n_params + n_outs)
    out_specs = (PartitionSpec("core"),) * n_outs
    f = jax.jit(
        shard_map(_body, mesh=mesh, in_specs=in_specs, out_specs=out_specs,
                  check_rep=False),
        donate_argnums=donate, keep_unused=True)
    return f, in_names, zero_shapes


def kernel(x, w1, b1, w2, b2, fw1, fb1, fw2, fb2):
    x = np.asarray(x, np.float32)
    w1 = np.asarray(w1, np.float32); b1 = np.asarray(b1, np.float32)
    w2 = np.asarray(w2, np.float32); b2 = np.asarray(b2, np.float32)

    devs = jax.devices()[:NCORES]
    mesh = Mesh(np.asarray(devs), ("core",))
    shard = NamedSharding(mesh, PartitionSpec("core"))

    # 1. async-put the big fixed tensor first: it transfers while we work
    fw1T = np.asarray(fw1, np.float32).T.astype(BF16)        # (32768, 1024)
    dev_in = {'fw1t': jax.device_put(fw1T, shard)}
    fb1g = np.zeros((NCORES, U), BF16)
    fb1g[0] = np.asarray(fb1, np.float32).astype(BF16)
    smalls = {
        'fb1r': fb1g,
        'fw2t': np.tile(np.ascontiguousarray(np.asarray(fw2, np.float32).T),
                        (NCORES, 1)),
        'fb2r': np.tile(np.asarray(fb2, np.float32).reshape(1, O2), (NCORES, 1)),
        'onesr': np.ones((NCORES, BL), np.float32),
        'onesb': np.ones((NCORES, 128), BF16),
        'ident': np.tile(np.eye(128, dtype=np.float32), (NCORES, 1)),
    }
    for k, v in smalls.items():
        dev_in[k] = jax.device_put(v, shard)

    # 2. bass + XLA compile in a side thread (neuronx-cc subprocess and the
    #    axon transfers overlap with the numpy conv work below)
    state = {}

    def _compile():
        if 'f' not in _CACHE:
            nc = _build_fc_kernel()
            f, in_names, zero_shapes = _jit_compile(nc, mesh)
            _CACHE['nc'] = nc
            _CACHE['f'] = f
            _CACHE['in_names'] = in_names
            _CACHE['zero_shapes'] = zero_shapes
        state['ready'] = True

    th = threading.Thread(target=_compile)
    th.start()

    # 3. host conv in batch quarters, each quarter's h2 shipped immediately
    h2q_dev = []
    for q in range(4):
        h2q = _host_convs_q(x[q*BQ:(q+1)*BQ], w1, b1, w2, b2)  # (BQ, 32768)
        h2q_dev.append(jax.device_put(h2q.T.astype(BF16), shard))

    th.join()
    f = _CACHE['f']
    in_names = _CACHE['in_names']
    zero_shapes = _CACHE['zero_shapes']

    # 4. four device calls (same executable), dispatched asynchronously
    results = []
    for q in range(4):
        args = []
        for name in in_names:
            args.append(h2q_dev[q] if name == 'h2t' else dev_in[name])
        zeros = [np.zeros((NCORES * s[0], *s[1:]), d) for s, d in zero_shapes]
        results.append(f(*args, *zeros))

    # 5. fetch + assemble: per call, core j returns batch rows q*BQ + j*BL ..
    out = np.empty((B, O2), np.float32)
    for q in range(4):
        og = np.asarray(results[q][0]).reshape(NCORES, O2, BL)
        for j in range(NCORES):
            out[q*BQ + j*BL:q*BQ + (j+1)*BL] = og[j].T
    return out


# revision 11
# speedup vs baseline: 16.6034x; 16.6034x over previous
"""Trainium2 kernel for nn_ConvNN_2D_Spatial_K_N_Location.

Strategy (8 NeuronCores):
  - The two KNN-conv layers (irregular top-9 selection, ~6% of FLOPs) run on
    host in fp32 with reference tie-breaking, using a candidate-projection
    table so per-token work is a 9-row cache-resident gather. The
    shuffle->unshuffle round trip between the layers cancels and is skipped.
  - The dominant fc1 (1024x32768x1024) is contraction-sharded across the 8
    cores: core j gets 1/8 of h2^T and 1/8 of fw1^T (bf16), so nothing is
    replicated over the slow host link. Partials are summed with an on-device
    ReduceScatter that also hands each core its 128-row batch shard for fc2.
  - Everything is pipelined: fw1 is device_put asynchronously up front, the
    batch is processed in 4 quarters (conv on host overlapping h2-quarter
    transfers and device calls), and the bass/XLA compile runs in a side
    thread while the first conv quarters compute.
"""
import threading
import numpy as np
import ml_dtypes

import jax
from jax.sharding import Mesh, NamedSharding, PartitionSpec
from jax.experimental.shard_map import shard_map

import concourse.bass as bass
import concourse.tile as tile
from concourse import bacc, mybir
from concourse import bass2jax as B2J

K, N, SCALE = 9, 8, 2
NCORES = 8
B = 1024
BQ = 256               # batch rows per device call (4 calls)
BL = BQ // NCORES      # 32 out rows per core per call
F = 32768              # fc1 contraction
FSH = F // NCORES      # 4096 per core
U = 1024               # fc1 output
O2 = 10                # final outputs

_CACHE = {}
BF16 = ml_dtypes.bfloat16


# ---------------------------------------------------------------- host conv
def _unshuffle(x, s):
    b, c, h, w = x.shape
    return x.reshape(b, c, h//s, s, w//s, s).transpose(0, 1, 3, 5, 2, 4).reshape(b, c*s*s, h//s, w//s)


def _shuffle(x, s):
    b, c, h, w = x.shape
    return x.reshape(b, c//(s*s), s, s, h, w).transpose(0, 1, 4, 2, 5, 3).reshape(b, c//(s*s), h*s, w*s)


def _conv_core(xc, w, bvec, H, W):
    """KNN conv on channel-major tokens. xc: (nb, C, H*W) -> (nb, Cout, H*W)."""
    nb, C, HW = xc.shape
    Cf = C + 2
    Cout = w.shape[0]
    xf = np.empty((nb, Cf, HW), np.float32)
    xf[:, :C] = xc
    gy, gx = np.meshgrid(np.linspace(0., 1., H, dtype=np.float32),
                         np.linspace(0., 1., W, dtype=np.float32), indexing='ij')
    xf[:, C] = gy.ravel()
    xf[:, C+1] = gx.ravel()
    ih = np.linspace(0, H-1, N).astype(np.int32)
    iw = np.linspace(0, W-1, N).astype(np.int32)
    cols = (ih[:, None] * W + iw[None, :]).ravel()
    samp = np.ascontiguousarray(xf[:, :, cols])            # (nb, Cf, 64)
    xt = np.ascontiguousarray(xf.transpose(0, 2, 1))       # (nb, HW, Cf)
    d2 = np.matmul(xt, samp)
    d2 *= -2.0
    d2 += np.einsum('bct,bct->bt', xf, xf)[:, :, None]
    d2 += np.einsum('bcn,bcn->bn', samp, samp)[:, None, :]
    # top-K nearest, ties toward lower candidate index (== jax top_k)
    idx = np.argsort(d2, axis=2, kind='stable')[:, :, :K]  # (nb, HW, K)
    # candidate projection table: Ptab[b, n, k, o] = sum_f samp[b,f,n] w[o,f,k]
    w2d = np.ascontiguousarray(w.transpose(1, 2, 0).reshape(Cf, K * Cout))
    st = np.ascontiguousarray(samp.transpose(0, 2, 1))     # (nb, 64, Cf)
    Ptab = np.matmul(st, w2d).reshape(nb, N*N, K, Cout)
    # chunk over images so the accumulator stays cache-resident
    out = np.empty((nb, HW, Cout), np.float32)
    CH = 16
    for c0 in range(0, nb, CH):
        sl = slice(c0, c0 + CH)
        Pc = Ptab[sl]
        ic = idx[sl]
        bi = np.arange(Pc.shape[0])[:, None]
        acc = Pc[bi, ic[:, :, 0], 0]                       # (CH, HW, Cout)
        for k in range(1, K):
            acc += Pc[bi, ic[:, :, k], k]
        out[sl] = acc
    out += bvec
    return np.ascontiguousarray(out.transpose(0, 2, 1))    # (nb, Cout, HW)


def _host_convs_q(xq, w1, b1, w2, b2):
    nb = xq.shape[0]
    a1 = _unshuffle(xq, SCALE).reshape(nb, 12, 256)
    o1 = _conv_core(a1, w1, b1, 16, 16)                    # (nb, 64, 256)
    np.maximum(o1, 0., out=o1)
    # shuffle -> unshuffle between the layers cancels exactly
    o2 = _conv_core(o1, w2, b2, 16, 16)                    # (nb, 128, 256)
    np.maximum(o2, 0., out=o2)
    return _shuffle(o2.reshape(nb, 128, 16, 16), SCALE).reshape(nb, F)


# ---------------------------------------------------------------- device fc
def _build_fc_kernel():
    nc = bacc.Bacc("TRN2", target_bir_lowering=False, debug=False,
                   enable_asserts=False, num_devices=NCORES)
    f32 = mybir.dt.float32
    bf16 = mybir.dt.bfloat16
    h2t = nc.dram_tensor("h2t", (FSH, BQ), bf16, kind="ExternalInput").ap()
    fw1t = nc.dram_tensor("fw1t", (FSH, U), bf16, kind="ExternalInput").ap()
    fb1r = nc.dram_tensor("fb1r", (1, U), bf16, kind="ExternalInput").ap()
    fw2t = nc.dram_tensor("fw2t", (U, O2), f32, kind="ExternalInput").ap()
    fb2r = nc.dram_tensor("fb2r", (1, O2), f32, kind="ExternalInput").ap()
    onesr = nc.dram_tensor("onesr", (1, BL), f32, kind="ExternalInput").ap()
    onesb = nc.dram_tensor("onesb", (1, 128), bf16, kind="ExternalInput").ap()
    ident = nc.dram_tensor("ident", (128, 128), f32, kind="ExternalInput").ap()
    outt = nc.dram_tensor("outt", (O2, BL), f32, kind="ExternalOutput").ap()

    NB = BQ // 128       # 2 batch blocks per call
    NF = FSH // 128      # 32 contraction tiles per core
    with tile.TileContext(nc) as tc:
        with tc.tile_pool(name="w", bufs=NF) as wpool, \
             tc.tile_pool(name="h", bufs=2 * NF) as hpool, \
             tc.tile_pool(name="small", bufs=1) as spool, \
             tc.tile_pool(name="acts", bufs=4) as apool, \
             tc.tile_pool(name="dram", bufs=1, space="DRAM") as dpool, \
             tc.tile_pool(name="ps", bufs=4, space="PSUM") as pspool, \
             tc.tile_pool(name="pst", bufs=2, space="PSUM") as ptpool:

            ones_t = spool.tile([1, BL], f32)
            nc.sync.dma_start(ones_t[:], onesr[:, :])
            onesb_t = spool.tile([1, 128], bf16)
            nc.sync.dma_start(onesb_t[:], onesb[:, :])
            fb1_t = spool.tile([1, U], bf16)
            nc.sync.dma_start(fb1_t[:], fb1r[:, :])
            fb2_t = spool.tile([1, O2], f32)
            nc.sync.dma_start(fb2_t[:], fb2r[:, :])
            id_t = spool.tile([128, 128], f32)
            nc.sync.dma_start(id_t[:], ident[:, :])
            fw2_t = spool.tile([128, 8 * O2], f32)
            for c in range(8):
                nc.sync.dma_start(fw2_t[:, bass.ts(c, O2)],
                                  fw2t[bass.ts(c, 128), :])

            # fw1 shard fully resident in SBUF (8 MB bf16)
            wt = []
            for i in range(NF):
                t = wpool.tile([128, U], bf16)
                nc.sync.dma_start(t[:], fw1t[bass.ts(i, 128), :])
                wt.append(t)

            cc_in = dpool.tile([BQ, U], f32)
            cc_out = dpool.tile([BL, U], f32)

            for bb in range(NB):
                ht = []
                for i in range(NF):
                    t = hpool.tile([128, 128], bf16)
                    nc.sync.dma_start(t[:], h2t[bass.ts(i, 128), bass.ts(bb, 128)])
                    ht.append(t)
                for uh in range(2):
                    ps = pspool.tile([128, 512], f32)
                    for i in range(NF):
                        nc.tensor.matmul(ps[:], lhsT=ht[i][:],
                                         rhs=wt[i][:, bass.ts(uh, 512)],
                                         start=(i == 0), stop=False)
                    # bias (only core 0's fb1r is nonzero)
                    nc.tensor.matmul(ps[:], lhsT=onesb_t[:],
                                     rhs=fb1_t[:, bass.ts(uh, 512)],
                                     start=False, stop=True)
                    pa = apool.tile([128, 512], f32)
                    nc.scalar.copy(pa[:], ps[:])
                    nc.sync.dma_start(cc_in[bass.ts(bb, 128), bass.ts(uh, 512)],
                                      pa[:])

            nc.gpsimd.collective_compute(
                "ReduceScatter", mybir.AluOpType.add,
                replica_groups=[list(range(NCORES))],
                ins=[cc_in.opt()], outs=[cc_out.opt()])

            h1 = apool.tile([BL, U], f32)
            nc.sync.dma_start(h1[:], cc_out[:])
            h1r = apool.tile([BL, U], f32)
            nc.scalar.activation(h1r[:], h1[:],
                                 mybir.ActivationFunctionType.Relu)

            # transpose h1r in (BL x 128) blocks (PE), then fc2
            h1T = apool.tile([128, 8 * BL], f32)
            for c in range(8):
                pt = ptpool.tile([128, BL], f32)
                nc.tensor.transpose(pt[:], h1r[:, bass.ts(c, 128)],
                                    id_t[0:BL, 0:BL])
                nc.scalar.copy(h1T[:, bass.ts(c, BL)], pt[:])

            psum2 = ptpool.tile([O2, BL], f32)
            for c in range(8):
                nc.tensor.matmul(psum2[:], lhsT=fw2_t[:, bass.ts(c, O2)],
                                 rhs=h1T[:, bass.ts(c, BL)],
                                 start=(c == 0), stop=False)
            nc.tensor.matmul(psum2[:], lhsT=fb2_t[:], rhs=ones_t[:],
                             start=False, stop=True)

            out_t = apool.tile([O2, BL], f32)
            nc.scalar.copy(out_t[:], psum2[:])
            nc.sync.dma_start(outt[:, :], out_t[:])

    nc.compile()
    return nc


def _jit_compile(nc, mesh):
    """Build + AOT-compile the sharded executable (run_bass_via_pjrt's path,
    without its host-side concat: we pass pre-sharded device arrays)."""
    B2J.install_neuronx_cc_hook()
    partition_name = nc.partition_id_tensor.name if nc.partition_id_tensor else None
    in_names, out_names, out_avals, zero_shapes = [], [], [], []
    for alloc in nc.m.functions[0].allocations:
        if not isinstance(alloc, mybir.MemoryLocationSet):
            continue
        name = alloc.memorylocations[0].name
        if alloc.kind == "ExternalInput":
            if name != partition_name:
                in_names.append(name)
        elif alloc.kind == "ExternalOutput":
            shape = tuple(alloc.tensor_shape)
            dtype = mybir.dt.np(alloc.dtype)
            out_names.append(name)
            out_avals.append(jax.core.ShapedArray(shape, dtype))
            zero_shapes.append((shape, dtype))
    n_params = len(in_names)
    n_outs = len(out_names)
    all_names = list(in_names) + list(out_names)
    if partition_name is not None:
        all_names.append(partition_name)

    def _body(*args):
        operands = list(args)
        if partition_name is not None:
            operands.append(B2J.partition_id_tensor())
        outs = B2J._bass_exec_p.bind(
            *operands,
            out_avals=tuple(out_avals),
            in_names=tuple(all_names),
            out_names=tuple(out_names),
            lowering_input_output_aliases=(),
            sim_require_finite=True,
            sim_require_nnan=True,
            nc=nc,
        )
        return tuple(outs)

    donate = tuple(range(n_params, n_params + n_outs))
    in_specs = (PartitionSpec("core"),) * (n_params + n_outs)
    out_specs = (PartitionSpec("core"),) * n_outs
    f = jax.jit(
        shard_map(_body, mesh=mesh, in_specs=in_specs, out_specs=out_specs,
                  check_rep=False),
        donate_argnums=donate, keep_unused=True)
    # AOT-compile now (XLA + neuronx-cc run outside the GIL, so this can
    # overlap with numpy work on another thread)
    shard = NamedSharding(mesh, PartitionSpec("core"))
    dram_in = {}
    for alloc in nc.m.functions[0].allocations:
        if isinstance(alloc, mybir.MemoryLocationSet) and alloc.kind == "ExternalInput":
            dram_in[alloc.memorylocations[0].name] = (
                tuple(alloc.tensor_shape), mybir.dt.np(alloc.dtype))
    specs = [jax.ShapeDtypeStruct((NCORES * dram_in[n][0][0], *dram_in[n][0][1:]),
                                  dram_in[n][1], sharding=shard)
             for n in in_names]
    specs += [jax.ShapeDtypeStruct((NCORES * s[0], *s[1:]), d, sharding=shard)
              for s, d in zero_shapes]
    compiled = f.lower(*specs).compile()
    return compiled, in_names, zero_shapes


def kernel(x, w1, b1, w2, b2, fw1, fb1, fw2, fb2):
    x = np.asarray(x, np.float32)
    w1 = np.asarray(w1, np.float32); b1 = np.asarray(b1, np.float32)
    w2 = np.asarray(w2, np.float32); b2 = np.asarray(b2, np.float32)

    devs = jax.devices()[:NCORES]
    mesh = Mesh(np.asarray(devs), ("core",))
    shard = NamedSharding(mesh, PartitionSpec("core"))

    # 1. async-put the big fixed tensor first: it transfers while we work
    fw1T = np.asarray(fw1, np.float32).T.astype(BF16)        # (32768, 1024)
    dev_in = {'fw1t': jax.device_put(fw1T, shard)}
    fb1g = np.zeros((NCORES, U), BF16)
    fb1g[0] = np.asarray(fb1, np.float32).astype(BF16)
    smalls = {
        'fb1r': fb1g,
        'fw2t': np.tile(np.ascontiguousarray(np.asarray(fw2, np.float32).T),
                        (NCORES, 1)),
        'fb2r': np.tile(np.asarray(fb2, np.float32).reshape(1, O2), (NCORES, 1)),
        'onesr': np.ones((NCORES, BL), np.float32),
        'onesb': np.ones((NCORES, 128), BF16),
        'ident': np.tile(np.eye(128, dtype=np.float32), (NCORES, 1)),
    }
    for k, v in smalls.items():
        dev_in[k] = jax.device_put(v, shard)

    # 2. bass + XLA compile in a side thread (neuronx-cc subprocess and the
    #    axon transfers overlap with the numpy conv work below)
    state = {}

    def _compile():
        try:
            if 'f' not in _CACHE:
                nc = _build_fc_kernel()
                f, in_names, zero_shapes = _jit_compile(nc, mesh)
                _CACHE['nc'] = nc
                _CACHE['f'] = f
                _CACHE['in_names'] = in_names
                _CACHE['zero_shapes'] = zero_shapes
            state['ready'] = True
        except BaseException as e:          # surfaced after join
            state['err'] = e

    th = threading.Thread(target=_compile)
    th.start()

    # 3. host conv in batch quarters, each quarter's h2 shipped immediately.
    #    Device calls are serialized (block call q-1 before dispatching q:
    #    concurrent ReduceScatters from different calls would interleave
    #    across cores) but overlap with the conv of later quarters.
    def _dispatch(q, h2q_dev):
        f = _CACHE['f']
        args = [h2q_dev if n == 'h2t' else dev_in[n] for n in _CACHE['in_names']]
        zeros = [np.zeros((NCORES * s[0], *s[1:]), d)
                 for s, d in _CACHE['zero_shapes']]
        return f(*args, *zeros)

    results = [None] * 4
    h2q_devs = [None] * 4
    last = -1
    for q in range(4):
        h2q = _host_convs_q(x[q*BQ:(q+1)*BQ], w1, b1, w2, b2)  # (BQ, 32768)
        h2q_devs[q] = jax.device_put(h2q.T.astype(BF16), shard)
        if 'ready' in state:
            while last + 1 <= q:
                if last >= 0:
                    jax.block_until_ready(results[last])
                last += 1
                results[last] = _dispatch(last, h2q_devs[last])

    th.join()
    if 'err' in state:
        raise state['err']
    for q in range(last + 1, 4):
        if q > 0:
            jax.block_until_ready(results[q - 1])
        results[q] = _dispatch(q, h2q_devs[q])

    # 5. fetch + assemble: per call, core j returns batch rows q*BQ + j*BL ..
    out = np.empty((B, O2), np.float32)
    for q in range(4):
        og = np.asarray(results[q][0]).reshape(NCORES, O2, BL)
        for j in range(NCORES):
            out[q*BQ + j*BL:q*BQ + (j+1)*BL] = og[j].T
    return out


# revision 13
# speedup vs baseline: 16.6948x; 1.0055x over previous
"""Trainium2 kernel for nn_ConvNN_2D_Spatial_K_N_Location.

Strategy (8 NeuronCores):
  - The two KNN-conv layers (irregular top-9 selection, ~6% of FLOPs) run on
    host in fp32 with reference tie-breaking, using a candidate-projection
    table so per-token work is a 9-row cache-resident gather. The
    shuffle->unshuffle round trip between the layers cancels and is skipped.
  - The dominant fc1 (1024x32768x1024) is contraction-sharded across the 8
    cores: core j gets 1/8 of h2^T and 1/8 of fw1^T (bf16), so nothing is
    replicated over the slow host link. Partials are summed with an on-device
    ReduceScatter that also hands each core its 128-row batch shard for fc2.
  - Everything is pipelined: fw1 is device_put asynchronously up front, the
    batch is processed in 4 quarters (conv on host overlapping h2-quarter
    transfers and device calls), and the bass/XLA compile runs in a side
    thread while the first conv quarters compute.
"""
import os
import threading
import time
import numpy as np
import ml_dtypes

_PROF = bool(os.environ.get("KPROF"))
_T0 = [0.0]


def _p(msg):
    if _PROF:
        print(f"[k {time.time()-_T0[0]:6.2f}] {msg}", flush=True)

import jax
from jax.sharding import Mesh, NamedSharding, PartitionSpec
from jax.experimental.shard_map import shard_map

import concourse.bass as bass
import concourse.tile as tile
from concourse import bacc, mybir
from concourse import bass2jax as B2J

K, N, SCALE = 9, 8, 2
NCORES = 8
B = 1024
BQ = 256               # batch rows per device call (4 calls)
BL = BQ // NCORES      # 32 out rows per core per call
F = 32768              # fc1 contraction
FSH = F // NCORES      # 4096 per core
U = 1024               # fc1 output
O2 = 10                # final outputs

_CACHE = {}
BF16 = ml_dtypes.bfloat16


# ---------------------------------------------------------------- host conv
def _unshuffle(x, s):
    b, c, h, w = x.shape
    return x.reshape(b, c, h//s, s, w//s, s).transpose(0, 1, 3, 5, 2, 4).reshape(b, c*s*s, h//s, w//s)


def _shuffle(x, s):
    b, c, h, w = x.shape
    return x.reshape(b, c//(s*s), s, s, h, w).transpose(0, 1, 4, 2, 5, 3).reshape(b, c//(s*s), h*s, w*s)


def _conv_core(xc, w, bvec, H, W):
    """KNN conv on channel-major tokens. xc: (nb, C, H*W) -> (nb, Cout, H*W)."""
    nb, C, HW = xc.shape
    Cf = C + 2
    Cout = w.shape[0]
    xf = np.empty((nb, Cf, HW), np.float32)
    xf[:, :C] = xc
    gy, gx = np.meshgrid(np.linspace(0., 1., H, dtype=np.float32),
                         np.linspace(0., 1., W, dtype=np.float32), indexing='ij')
    xf[:, C] = gy.ravel()
    xf[:, C+1] = gx.ravel()
    ih = np.linspace(0, H-1, N).astype(np.int32)
    iw = np.linspace(0, W-1, N).astype(np.int32)
    cols = (ih[:, None] * W + iw[None, :]).ravel()
    samp = np.ascontiguousarray(xf[:, :, cols])            # (nb, Cf, 64)
    xt = np.ascontiguousarray(xf.transpose(0, 2, 1))       # (nb, HW, Cf)
    d2 = np.matmul(xt, samp)
    d2 *= -2.0
    d2 += np.einsum('bct,bct->bt', xf, xf)[:, :, None]
    d2 += np.einsum('bcn,bcn->bn', samp, samp)[:, None, :]
    # top-K nearest, ties toward lower candidate index (== jax top_k)
    idx = np.argsort(d2, axis=2, kind='stable')[:, :, :K]  # (nb, HW, K)
    # candidate projection table: Ptab[b, n, k, o] = sum_f samp[b,f,n] w[o,f,k]
    w2d = np.ascontiguousarray(w.transpose(1, 2, 0).reshape(Cf, K * Cout))
    st = np.ascontiguousarray(samp.transpose(0, 2, 1))     # (nb, 64, Cf)
    Ptab = np.matmul(st, w2d).reshape(nb, N*N, K, Cout)
    # chunk over images so the accumulator stays cache-resident
    out = np.empty((nb, HW, Cout), np.float32)
    CH = 16
    for c0 in range(0, nb, CH):
        sl = slice(c0, c0 + CH)
        Pc = Ptab[sl]
        ic = idx[sl]
        bi = np.arange(Pc.shape[0])[:, None]
        acc = Pc[bi, ic[:, :, 0], 0]                       # (CH, HW, Cout)
        for k in range(1, K):
            acc += Pc[bi, ic[:, :, k], k]
        out[sl] = acc
    out += bvec
    return np.ascontiguousarray(out.transpose(0, 2, 1))    # (nb, Cout, HW)


def _host_convs_q(xq, w1, b1, w2, b2):
    nb = xq.shape[0]
    a1 = _unshuffle(xq, SCALE).reshape(nb, 12, 256)
    o1 = _conv_core(a1, w1, b1, 16, 16)                    # (nb, 64, 256)
    np.maximum(o1, 0., out=o1)
    # shuffle -> unshuffle between the layers cancels exactly
    o2 = _conv_core(o1, w2, b2, 16, 16)                    # (nb, 128, 256)
    np.maximum(o2, 0., out=o2)
    return _shuffle(o2.reshape(nb, 128, 16, 16), SCALE).reshape(nb, F)


# ---------------------------------------------------------------- device fc
def _build_fc_kernel():
    nc = bacc.Bacc("TRN2", target_bir_lowering=False, debug=False,
                   enable_asserts=False, num_devices=NCORES)
    f32 = mybir.dt.float32
    bf16 = mybir.dt.bfloat16
    h2t = nc.dram_tensor("h2t", (FSH, BQ), bf16, kind="ExternalInput").ap()
    fw1t = nc.dram_tensor("fw1t", (FSH, U), bf16, kind="ExternalInput").ap()
    fb1r = nc.dram_tensor("fb1r", (1, U), bf16, kind="ExternalInput").ap()
    fw2t = nc.dram_tensor("fw2t", (U, O2), f32, kind="ExternalInput").ap()
    fb2r = nc.dram_tensor("fb2r", (1, O2), f32, kind="ExternalInput").ap()
    onesr = nc.dram_tensor("onesr", (1, BL), f32, kind="ExternalInput").ap()
    onesb = nc.dram_tensor("onesb", (1, 128), bf16, kind="ExternalInput").ap()
    ident = nc.dram_tensor("ident", (128, 128), f32, kind="ExternalInput").ap()
    outt = nc.dram_tensor("outt", (O2, BL), f32, kind="ExternalOutput").ap()

    NB = BQ // 128       # 2 batch blocks per call
    NF = FSH // 128      # 32 contraction tiles per core
    with tile.TileContext(nc) as tc:
        with tc.tile_pool(name="w", bufs=NF) as wpool, \
             tc.tile_pool(name="h", bufs=2 * NF) as hpool, \
             tc.tile_pool(name="small", bufs=1) as spool, \
             tc.tile_pool(name="acts", bufs=4) as apool, \
             tc.tile_pool(name="dram", bufs=1, space="DRAM") as dpool, \
             tc.tile_pool(name="ps", bufs=4, space="PSUM") as pspool, \
             tc.tile_pool(name="pst", bufs=2, space="PSUM") as ptpool:

            ones_t = spool.tile([1, BL], f32)
            nc.sync.dma_start(ones_t[:], onesr[:, :])
            onesb_t = spool.tile([1, 128], bf16)
            nc.sync.dma_start(onesb_t[:], onesb[:, :])
            fb1_t = spool.tile([1, U], bf16)
            nc.sync.dma_start(fb1_t[:], fb1r[:, :])
            fb2_t = spool.tile([1, O2], f32)
            nc.sync.dma_start(fb2_t[:], fb2r[:, :])
            id_t = spool.tile([128, 128], f32)
            nc.sync.dma_start(id_t[:], ident[:, :])
            fw2_t = spool.tile([128, 8 * O2], f32)
            for c in range(8):
                nc.sync.dma_start(fw2_t[:, bass.ts(c, O2)],
                                  fw2t[bass.ts(c, 128), :])

            # fw1 shard fully resident in SBUF (8 MB bf16)
            wt = []
            for i in range(NF):
                t = wpool.tile([128, U], bf16)
                nc.sync.dma_start(t[:], fw1t[bass.ts(i, 128), :])
                wt.append(t)

            cc_in = dpool.tile([BQ, U], f32)
            cc_out = dpool.tile([BL, U], f32)

            for bb in range(NB):
                ht = []
                for i in range(NF):
                    t = hpool.tile([128, 128], bf16)
                    nc.sync.dma_start(t[:], h2t[bass.ts(i, 128), bass.ts(bb, 128)])
                    ht.append(t)
                for uh in range(2):
                    ps = pspool.tile([128, 512], f32)
                    for i in range(NF):
                        nc.tensor.matmul(ps[:], lhsT=ht[i][:],
                                         rhs=wt[i][:, bass.ts(uh, 512)],
                                         start=(i == 0), stop=False)
                    # bias (only core 0's fb1r is nonzero)
                    nc.tensor.matmul(ps[:], lhsT=onesb_t[:],
                                     rhs=fb1_t[:, bass.ts(uh, 512)],
                                     start=False, stop=True)
                    pa = apool.tile([128, 512], f32)
                    nc.scalar.copy(pa[:], ps[:])
                    nc.sync.dma_start(cc_in[bass.ts(bb, 128), bass.ts(uh, 512)],
                                      pa[:])

            nc.gpsimd.collective_compute(
                "ReduceScatter", mybir.AluOpType.add,
                replica_groups=[list(range(NCORES))],
                ins=[cc_in.opt()], outs=[cc_out.opt()])

            h1 = apool.tile([BL, U], f32)
            nc.sync.dma_start(h1[:], cc_out[:])
            h1r = apool.tile([BL, U], f32)
            nc.scalar.activation(h1r[:], h1[:],
                                 mybir.ActivationFunctionType.Relu)

            # transpose h1r in (BL x 128) blocks (PE), then fc2
            h1T = apool.tile([128, 8 * BL], f32)
            for c in range(8):
                pt = ptpool.tile([128, BL], f32)
                nc.tensor.transpose(pt[:], h1r[:, bass.ts(c, 128)],
                                    id_t[0:BL, 0:BL])
                nc.scalar.copy(h1T[:, bass.ts(c, BL)], pt[:])

            psum2 = ptpool.tile([O2, BL], f32)
            for c in range(8):
                nc.tensor.matmul(psum2[:], lhsT=fw2_t[:, bass.ts(c, O2)],
                                 rhs=h1T[:, bass.ts(c, BL)],
                                 start=(c == 0), stop=False)
            nc.tensor.matmul(psum2[:], lhsT=fb2_t[:], rhs=ones_t[:],
                             start=False, stop=True)

            out_t = apool.tile([O2, BL], f32)
            nc.scalar.copy(out_t[:], psum2[:])
            nc.sync.dma_start(outt[:, :], out_t[:])

    nc.compile()
    return nc


def _jit_compile(nc, mesh):
    """Build + AOT-compile the sharded executable (run_bass_via_pjrt's path,
    without its host-side concat: we pass pre-sharded device arrays)."""
    B2J.install_neuronx_cc_hook()
    partition_name = nc.partition_id_tensor.name if nc.partition_id_tensor else None
    in_names, out_names, out_avals, zero_shapes = [], [], [], []
    for alloc in nc.m.functions[0].allocations:
        if not isinstance(alloc, mybir.MemoryLocationSet):
            continue
        name = alloc.memorylocations[0].name
        if alloc.kind == "ExternalInput":
            if name != partition_name:
                in_names.append(name)
        elif alloc.kind == "ExternalOutput":
            shape = tuple(alloc.tensor_shape)
            dtype = mybir.dt.np(alloc.dtype)
            out_names.append(name)
            out_avals.append(jax.core.ShapedArray(shape, dtype))
            zero_shapes.append((shape, dtype))
    n_params = len(in_names)
    n_outs = len(out_names)
    all_names = list(in_names) + list(out_names)
    if partition_name is not None:
        all_names.append(partition_name)

    def _body(*args):
        operands = list(args)
        if partition_name is not None:
            operands.append(B2J.partition_id_tensor())
        outs = B2J._bass_exec_p.bind(
            *operands,
            out_avals=tuple(out_avals),
            in_names=tuple(all_names),
            out_names=tuple(out_names),
            lowering_input_output_aliases=(),
            sim_require_finite=True,
            sim_require_nnan=True,
            nc=nc,
        )
        return tuple(outs)

    donate = tuple(range(n_params, n_params + n_outs))
    in_specs = (PartitionSpec("core"),) * (n_params + n_outs)
    out_specs = (PartitionSpec("core"),) * n_outs
    f = jax.jit(
        shard_map(_body, mesh=mesh, in_specs=in_specs, out_specs=out_specs,
                  check_rep=False),
        donate_argnums=donate, keep_unused=True)
    # AOT-compile now (XLA + neuronx-cc run outside the GIL, so this can
    # overlap with numpy work on another thread)
    shard = NamedSharding(mesh, PartitionSpec("core"))
    dram_in = {}
    for alloc in nc.m.functions[0].allocations:
        if isinstance(alloc, mybir.MemoryLocationSet) and alloc.kind == "ExternalInput":
            dram_in[alloc.memorylocations[0].name] = (
                tuple(alloc.tensor_shape), mybir.dt.np(alloc.dtype))
    specs = [jax.ShapeDtypeStruct((NCORES * dram_in[n][0][0], *dram_in[n][0][1:]),
                                  dram_in[n][1], sharding=shard)
             for n in in_names]
    specs += [jax.ShapeDtypeStruct((NCORES * s[0], *s[1:]), d, sharding=shard)
              for s, d in zero_shapes]
    compiled = f.lower(*specs).compile()
    return compiled, in_names, zero_shapes


def kernel(x, w1, b1, w2, b2, fw1, fb1, fw2, fb2):
    _T0[0] = time.time()
    x = np.asarray(x, np.float32)
    w1 = np.asarray(w1, np.float32); b1 = np.asarray(b1, np.float32)
    w2 = np.asarray(w2, np.float32); b2 = np.asarray(b2, np.float32)

    devs = jax.devices()[:NCORES]
    mesh = Mesh(np.asarray(devs), ("core",))
    shard = NamedSharding(mesh, PartitionSpec("core"))

    # 1. async-put the big fixed tensor first: it transfers while we work
    fw1T = np.asarray(fw1, np.float32).T.astype(BF16)        # (32768, 1024)
    dev_in = {'fw1t': jax.device_put(fw1T, shard)}
    fb1g = np.zeros((NCORES, U), BF16)
    fb1g[0] = np.asarray(fb1, np.float32).astype(BF16)
    smalls = {
        'fb1r': fb1g,
        'fw2t': np.tile(np.ascontiguousarray(np.asarray(fw2, np.float32).T),
                        (NCORES, 1)),
        'fb2r': np.tile(np.asarray(fb2, np.float32).reshape(1, O2), (NCORES, 1)),
        'onesr': np.ones((NCORES, BL), np.float32),
        'onesb': np.ones((NCORES, 128), BF16),
        'ident': np.tile(np.eye(128, dtype=np.float32), (NCORES, 1)),
    }
    for k, v in smalls.items():
        dev_in[k] = jax.device_put(v, shard)
    _p("puts launched")

    # 2. bass + XLA compile in a side thread (neuronx-cc subprocess and the
    #    axon transfers overlap with the numpy conv work below)
    state = {}

    def _compile():
        try:
            if 'f' not in _CACHE:
                nc = _build_fc_kernel()
                f, in_names, zero_shapes = _jit_compile(nc, mesh)
                _CACHE['nc'] = nc
                _CACHE['f'] = f
                _CACHE['in_names'] = in_names
                _CACHE['zero_shapes'] = zero_shapes
            state['ready'] = True
            _p("compile thread done")
        except BaseException as e:          # surfaced after join
            state['err'] = e

    th = threading.Thread(target=_compile)
    th.start()

    # 3. host conv in batch quarters, each quarter's h2 shipped immediately.
    #    Device calls are serialized (block call q-1 before dispatching q:
    #    concurrent ReduceScatters from different calls would interleave
    #    across cores) but overlap with the conv of later quarters.
    def _dispatch(q, h2q_dev):
        f = _CACHE['f']
        args = [h2q_dev if n == 'h2t' else dev_in[n] for n in _CACHE['in_names']]
        zeros = [np.zeros((NCORES * s[0], *s[1:]), d)
                 for s, d in _CACHE['zero_shapes']]
        return f(*args, *zeros)

    results = [None] * 4
    h2q_devs = [None] * 4
    last = -1
    for q in range(4):
        h2q = _host_convs_q(x[q*BQ:(q+1)*BQ], w1, b1, w2, b2)  # (BQ, 32768)
        _p(f"conv q{q} done")
        h2q_devs[q] = jax.device_put(h2q.T.astype(BF16), shard)
        if 'ready' in state:
            while last + 1 <= q:
                if last >= 0:
                    jax.block_until_ready(results[last])
                last += 1
                results[last] = _dispatch(last, h2q_devs[last])
                _p(f"dispatched q{last}")

    th.join()
    if 'err' in state:
        raise state['err']
    for q in range(last + 1, 4):
        if q > 0:
            jax.block_until_ready(results[q - 1])
        results[q] = _dispatch(q, h2q_devs[q])
        _p(f"dispatched q{q} (post-join)")

    # 5. fetch + assemble: per call, core j returns batch rows q*BQ + j*BL ..
    out = np.empty((B, O2), np.float32)
    for q in range(4):
        og = np.asarray(results[q][0]).reshape(NCORES, O2, BL)
        _p(f"fetched q{q}")
        for j in range(NCORES):
            out[q*BQ + j*BL:q*BQ + (j+1)*BL] = og[j].T
    return out


# revision 16
# speedup vs baseline: 17.3421x; 1.0388x over previous
"""Trainium2 kernel for nn_ConvNN_2D_Spatial_K_N_Location.

Strategy (8 NeuronCores):
  - The two KNN-conv layers (irregular top-9 selection, ~6% of FLOPs) run on
    host in fp32 with reference tie-breaking, using a candidate-projection
    table so per-token work is a 9-row cache-resident gather. The
    shuffle->unshuffle round trip between the layers cancels and is skipped.
  - The dominant fc1 (1024x32768x1024) is contraction-sharded across the 8
    cores: core j gets 1/8 of h2^T and 1/8 of fw1^T (bf16), so nothing is
    replicated over the slow host link. Partials are summed with an on-device
    ReduceScatter that also hands each core its 128-row batch shard for fc2.
  - Everything is pipelined: fw1 is device_put asynchronously up front, the
    batch is processed in 4 quarters (conv on host overlapping h2-quarter
    transfers and device calls), and the bass/XLA compile runs in a side
    thread while the first conv quarters compute.
"""
import os
import threading
import time
import numpy as np
import ml_dtypes

_PROF = bool(os.environ.get("KPROF"))
_T0 = [0.0]


def _p(msg):
    if _PROF:
        print(f"[k {time.time()-_T0[0]:6.2f}] {msg}", flush=True)

import jax
from jax.sharding import Mesh, NamedSharding, PartitionSpec
from jax.experimental.shard_map import shard_map

import concourse.bass as bass
import concourse.tile as tile
from concourse import bacc, mybir
from concourse import bass2jax as B2J

K, N, SCALE = 9, 8, 2
NCORES = 8
B = 1024
BQ = 256               # batch rows per device call (4 calls)
BL = BQ // NCORES      # 32 out rows per core per call
F = 32768              # fc1 contraction
FSH = F // NCORES      # 4096 per core
U = 1024               # fc1 output
O2 = 10                # final outputs

_CACHE = {}
BF16 = ml_dtypes.bfloat16


# ---------------------------------------------------------------- host conv
def _unshuffle(x, s):
    b, c, h, w = x.shape
    return x.reshape(b, c, h//s, s, w//s, s).transpose(0, 1, 3, 5, 2, 4).reshape(b, c*s*s, h//s, w//s)


def _shuffle(x, s):
    b, c, h, w = x.shape
    return x.reshape(b, c//(s*s), s, s, h, w).transpose(0, 1, 4, 2, 5, 3).reshape(b, c//(s*s), h*s, w*s)


def _conv_core(xc, w, bvec, H, W):
    """KNN conv on channel-major tokens. xc: (nb, C, H*W) -> (nb, Cout, H*W)."""
    nb, C, HW = xc.shape
    Cf = C + 2
    Cout = w.shape[0]
    xf = np.empty((nb, Cf, HW), np.float32)
    xf[:, :C] = xc
    gy, gx = np.meshgrid(np.linspace(0., 1., H, dtype=np.float32),
                         np.linspace(0., 1., W, dtype=np.float32), indexing='ij')
    xf[:, C] = gy.ravel()
    xf[:, C+1] = gx.ravel()
    ih = np.linspace(0, H-1, N).astype(np.int32)
    iw = np.linspace(0, W-1, N).astype(np.int32)
    cols = (ih[:, None] * W + iw[None, :]).ravel()
    samp = np.ascontiguousarray(xf[:, :, cols])            # (nb, Cf, 64)
    xt = np.ascontiguousarray(xf.transpose(0, 2, 1))       # (nb, HW, Cf)
    d2 = np.matmul(xt, samp)
    d2 *= -2.0
    d2 += np.einsum('bct,bct->bt', xf, xf)[:, :, None]
    d2 += np.einsum('bcn,bcn->bn', samp, samp)[:, None, :]
    # top-K nearest, ties toward lower candidate index (== jax top_k)
    idx = np.argsort(d2, axis=2, kind='stable')[:, :, :K]  # (nb, HW, K)
    # candidate projection table: Ptab[b, n, k, o] = sum_f samp[b,f,n] w[o,f,k]
    w2d = np.ascontiguousarray(w.transpose(1, 2, 0).reshape(Cf, K * Cout))
    st = np.ascontiguousarray(samp.transpose(0, 2, 1))     # (nb, 64, Cf)
    Ptab = np.matmul(st, w2d).reshape(nb, N*N, K, Cout)
    # chunk over images so the accumulator stays cache-resident
    out = np.empty((nb, HW, Cout), np.float32)
    CH = 16
    for c0 in range(0, nb, CH):
        sl = slice(c0, c0 + CH)
        Pc = Ptab[sl]
        ic = idx[sl]
        bi = np.arange(Pc.shape[0])[:, None]
        acc = Pc[bi, ic[:, :, 0], 0]                       # (CH, HW, Cout)
        for k in range(1, K):
            acc += Pc[bi, ic[:, :, k], k]
        out[sl] = acc
    out += bvec
    return np.ascontiguousarray(out.transpose(0, 2, 1))    # (nb, Cout, HW)


def _host_convs_q(xq, w1, b1, w2, b2):
    nb = xq.shape[0]
    a1 = _unshuffle(xq, SCALE).reshape(nb, 12, 256)
    o1 = _conv_core(a1, w1, b1, 16, 16)                    # (nb, 64, 256)
    np.maximum(o1, 0., out=o1)
    # shuffle -> unshuffle between the layers cancels exactly
    o2 = _conv_core(o1, w2, b2, 16, 16)                    # (nb, 128, 256)
    np.maximum(o2, 0., out=o2)
    return _shuffle(o2.reshape(nb, 128, 16, 16), SCALE).reshape(nb, F)


# ---------------------------------------------------------------- device fc
def _build_fc_kernel():
    nc = bacc.Bacc("TRN2", target_bir_lowering=False, debug=False,
                   enable_asserts=False, num_devices=NCORES)
    f32 = mybir.dt.float32
    bf16 = mybir.dt.bfloat16
    h2t = nc.dram_tensor("h2t", (FSH, BQ), bf16, kind="ExternalInput").ap()
    fw1t = nc.dram_tensor("fw1t", (FSH, U), bf16, kind="ExternalInput").ap()
    fb1r = nc.dram_tensor("fb1r", (1, U), bf16, kind="ExternalInput").ap()
    fw2t = nc.dram_tensor("fw2t", (U, O2), f32, kind="ExternalInput").ap()
    fb2r = nc.dram_tensor("fb2r", (1, O2), f32, kind="ExternalInput").ap()
    onesr = nc.dram_tensor("onesr", (1, BL), f32, kind="ExternalInput").ap()
    onesb = nc.dram_tensor("onesb", (1, 128), bf16, kind="ExternalInput").ap()
    ident = nc.dram_tensor("ident", (128, 128), f32, kind="ExternalInput").ap()
    outt = nc.dram_tensor("outt", (O2, BL), f32, kind="ExternalOutput").ap()

    NB = BQ // 128       # 2 batch blocks per call
    NF = FSH // 128      # 32 contraction tiles per core
    with tile.TileContext(nc) as tc:
        with tc.tile_pool(name="w", bufs=NF) as wpool, \
             tc.tile_pool(name="h", bufs=2 * NF) as hpool, \
             tc.tile_pool(name="small", bufs=1) as spool, \
             tc.tile_pool(name="acts", bufs=4) as apool, \
             tc.tile_pool(name="dram", bufs=1, space="DRAM") as dpool, \
             tc.tile_pool(name="ps", bufs=4, space="PSUM") as pspool, \
             tc.tile_pool(name="pst", bufs=2, space="PSUM") as ptpool:

            ones_t = spool.tile([1, BL], f32)
            nc.sync.dma_start(ones_t[:], onesr[:, :])
            onesb_t = spool.tile([1, 128], bf16)
            nc.sync.dma_start(onesb_t[:], onesb[:, :])
            fb1_t = spool.tile([1, U], bf16)
            nc.sync.dma_start(fb1_t[:], fb1r[:, :])
            fb2_t = spool.tile([1, O2], f32)
            nc.sync.dma_start(fb2_t[:], fb2r[:, :])
            id_t = spool.tile([128, 128], f32)
            nc.sync.dma_start(id_t[:], ident[:, :])
            fw2_t = spool.tile([128, 8 * O2], f32)
            for c in range(8):
                nc.sync.dma_start(fw2_t[:, bass.ts(c, O2)],
                                  fw2t[bass.ts(c, 128), :])

            # fw1 shard fully resident in SBUF (8 MB bf16)
            wt = []
            for i in range(NF):
                t = wpool.tile([128, U], bf16)
                nc.sync.dma_start(t[:], fw1t[bass.ts(i, 128), :])
                wt.append(t)

            cc_in = dpool.tile([BQ, U], f32)
            cc_out = dpool.tile([BL, U], f32)

            for bb in range(NB):
                ht = []
                for i in range(NF):
                    t = hpool.tile([128, 128], bf16)
                    nc.sync.dma_start(t[:], h2t[bass.ts(i, 128), bass.ts(bb, 128)])
                    ht.append(t)
                for uh in range(2):
                    ps = pspool.tile([128, 512], f32)
                    for i in range(NF):
                        nc.tensor.matmul(ps[:], lhsT=ht[i][:],
                                         rhs=wt[i][:, bass.ts(uh, 512)],
                                         start=(i == 0), stop=False)
                    # bias (only core 0's fb1r is nonzero)
                    nc.tensor.matmul(ps[:], lhsT=onesb_t[:],
                                     rhs=fb1_t[:, bass.ts(uh, 512)],
                                     start=False, stop=True)
                    pa = apool.tile([128, 512], f32)
                    nc.scalar.copy(pa[:], ps[:])
                    nc.sync.dma_start(cc_in[bass.ts(bb, 128), bass.ts(uh, 512)],
                                      pa[:])

            nc.gpsimd.collective_compute(
                "ReduceScatter", mybir.AluOpType.add,
                replica_groups=[list(range(NCORES))],
                ins=[cc_in.opt()], outs=[cc_out.opt()])

            h1 = apool.tile([BL, U], f32)
            nc.sync.dma_start(h1[:], cc_out[:])
            h1r = apool.tile([BL, U], f32)
            nc.scalar.activation(h1r[:], h1[:],
                                 mybir.ActivationFunctionType.Relu)

            # transpose h1r in (BL x 128) blocks (PE), then fc2
            h1T = apool.tile([128, 8 * BL], f32)
            for c in range(8):
                pt = ptpool.tile([128, BL], f32)
                nc.tensor.transpose(pt[:], h1r[:, bass.ts(c, 128)],
                                    id_t[0:BL, 0:BL])
                nc.scalar.copy(h1T[:, bass.ts(c, BL)], pt[:])

            psum2 = ptpool.tile([O2, BL], f32)
            for c in range(8):
                nc.tensor.matmul(psum2[:], lhsT=fw2_t[:, bass.ts(c, O2)],
                                 rhs=h1T[:, bass.ts(c, BL)],
                                 start=(c == 0), stop=False)
            nc.tensor.matmul(psum2[:], lhsT=fb2_t[:], rhs=ones_t[:],
                             start=False, stop=True)

            out_t = apool.tile([O2, BL], f32)
            nc.scalar.copy(out_t[:], psum2[:])
            nc.sync.dma_start(outt[:, :], out_t[:])

    nc.compile()
    return nc


def _jit_compile(nc, mesh):
    """Build + AOT-compile the sharded executable (run_bass_via_pjrt's path,
    without its host-side concat: we pass pre-sharded device arrays)."""
    B2J.install_neuronx_cc_hook()
    partition_name = nc.partition_id_tensor.name if nc.partition_id_tensor else None
    in_names, out_names, out_avals, zero_shapes = [], [], [], []
    for alloc in nc.m.functions[0].allocations:
        if not isinstance(alloc, mybir.MemoryLocationSet):
            continue
        name = alloc.memorylocations[0].name
        if alloc.kind == "ExternalInput":
            if name != partition_name:
                in_names.append(name)
        elif alloc.kind == "ExternalOutput":
            shape = tuple(alloc.tensor_shape)
            dtype = mybir.dt.np(alloc.dtype)
            out_names.append(name)
            out_avals.append(jax.core.ShapedArray(shape, dtype))
            zero_shapes.append((shape, dtype))
    n_params = len(in_names)
    n_outs = len(out_names)
    all_names = list(in_names) + list(out_names)
    if partition_name is not None:
        all_names.append(partition_name)

    def _body(*args):
        operands = list(args)
        if partition_name is not None:
            operands.append(B2J.partition_id_tensor())
        outs = B2J._bass_exec_p.bind(
            *operands,
            out_avals=tuple(out_avals),
            in_names=tuple(all_names),
            out_names=tuple(out_names),
            lowering_input_output_aliases=(),
            sim_require_finite=True,
            sim_require_nnan=True,
            nc=nc,
        )
        return tuple(outs)

    donate = tuple(range(n_params, n_params + n_outs))
    in_specs = (PartitionSpec("core"),) * (n_params + n_outs)
    out_specs = (PartitionSpec("core"),) * n_outs
    f = jax.jit(
        shard_map(_body, mesh=mesh, in_specs=in_specs, out_specs=out_specs,
                  check_rep=False),
        donate_argnums=donate, keep_unused=True)
    # AOT-compile now (XLA + neuronx-cc run outside the GIL, so this can
    # overlap with numpy work on another thread)
    shard = NamedSharding(mesh, PartitionSpec("core"))
    dram_in = {}
    for alloc in nc.m.functions[0].allocations:
        if isinstance(alloc, mybir.MemoryLocationSet) and alloc.kind == "ExternalInput":
            dram_in[alloc.memorylocations[0].name] = (
                tuple(alloc.tensor_shape), mybir.dt.np(alloc.dtype))
    specs = [jax.ShapeDtypeStruct((NCORES * dram_in[n][0][0], *dram_in[n][0][1:]),
                                  dram_in[n][1], sharding=shard)
             for n in in_names]
    specs += [jax.ShapeDtypeStruct((NCORES * s[0], *s[1:]), d, sharding=shard)
              for s, d in zero_shapes]
    compiled = f.lower(*specs).compile()
    return compiled, in_names, zero_shapes


def _put_sharded(arr, devs, mesh):
    """Async row-sharded put: per-device slices (NamedSharding device_put
    serializes; this overlaps all 8 transfers with later host work)."""
    n = len(devs)
    rows = arr.shape[0] // n
    shard = NamedSharding(mesh, PartitionSpec("core"))
    pieces = [jax.device_put(arr[j*rows:(j+1)*rows], devs[j]) for j in range(n)]
    return jax.make_array_from_single_device_arrays(arr.shape, shard, pieces)


def kernel(x, w1, b1, w2, b2, fw1, fb1, fw2, fb2):
    _T0[0] = time.time()
    x = np.asarray(x, np.float32)
    w1 = np.asarray(w1, np.float32); b1 = np.asarray(b1, np.float32)
    w2 = np.asarray(w2, np.float32); b2 = np.asarray(b2, np.float32)

    devs = jax.devices()[:NCORES]
    mesh = Mesh(np.asarray(devs), ("core",))
    shard = NamedSharding(mesh, PartitionSpec("core"))

    # 1. async-put the big fixed tensor first: it transfers while we work
    fw1T = np.asarray(fw1, np.float32).T.astype(BF16)        # (32768, 1024)
    dev_in = {'fw1t': _put_sharded(fw1T, devs, mesh)}
    fb1g = np.zeros((NCORES, U), BF16)
    fb1g[0] = np.asarray(fb1, np.float32).astype(BF16)
    smalls = {
        'fb1r': fb1g,
        'fw2t': np.tile(np.ascontiguousarray(np.asarray(fw2, np.float32).T),
                        (NCORES, 1)),
        'fb2r': np.tile(np.asarray(fb2, np.float32).reshape(1, O2), (NCORES, 1)),
        'onesr': np.ones((NCORES, BL), np.float32),
        'onesb': np.ones((NCORES, 128), BF16),
        'ident': np.tile(np.eye(128, dtype=np.float32), (NCORES, 1)),
    }
    for k, v in smalls.items():
        dev_in[k] = _put_sharded(v, devs, mesh)
    _p("puts launched")

    # 2. bass + XLA compile in a side thread (neuronx-cc subprocess and the
    #    axon transfers overlap with the numpy conv work below)
    state = {}

    def _compile():
        try:
            if 'f' not in _CACHE:
                nc = _build_fc_kernel()
                f, in_names, zero_shapes = _jit_compile(nc, mesh)
                _CACHE['nc'] = nc
                _CACHE['f'] = f
                _CACHE['in_names'] = in_names
                _CACHE['zero_shapes'] = zero_shapes
            state['ready'] = True
            _p("compile thread done")
        except BaseException as e:          # surfaced after join
            state['err'] = e

    th = threading.Thread(target=_compile)
    th.start()

    # 3. host conv in batch quarters, each quarter's h2 shipped immediately.
    #    Device calls are serialized (block call q-1 before dispatching q:
    #    concurrent ReduceScatters from different calls would interleave
    #    across cores) but overlap with the conv of later quarters.
    zeros_q = [None] * 4

    def _dispatch(q, h2q_dev):
        f = _CACHE['f']
        args = [h2q_dev if n == 'h2t' else dev_in[n] for n in _CACHE['in_names']]
        return f(*args, *zeros_q[q])

    results = [None] * 4
    h2q_devs = [None] * 4
    last = -1
    for q in range(4):
        zeros_q[q] = [_put_sharded(np.zeros((NCORES * s[0], *s[1:]), d),
                                   devs, mesh)
                      for s, d in (_CACHE.get('zero_shapes') or [((O2, BL),
                                                                  np.float32)])]
        h2q = _host_convs_q(x[q*BQ:(q+1)*BQ], w1, b1, w2, b2)  # (BQ, 32768)
        _p(f"conv q{q} done")
        h2q_devs[q] = _put_sharded(h2q.T.astype(BF16), devs, mesh)
        if 'ready' in state:
            while last + 1 <= q:
                if last >= 0:
                    jax.block_until_ready(results[last])
                last += 1
                results[last] = _dispatch(last, h2q_devs[last])
                _p(f"dispatched q{last}")

    th.join()
    if 'err' in state:
        raise state['err']
    for q in range(last + 1, 4):
        if q > 0:
            jax.block_until_ready(results[q - 1])
        results[q] = _dispatch(q, h2q_devs[q])
        _p(f"dispatched q{q} (post-join)")

    # 5. fetch + assemble: per call, core j returns batch rows q*BQ + j*BL ..
    out = np.empty((B, O2), np.float32)
    for q in range(4):
        og = np.asarray(results[q][0]).reshape(NCORES, O2, BL)
        _p(f"fetched q{q}")
        for j in range(NCORES):
            out[q*BQ + j*BL:q*BQ + (j+1)*BL] = og[j].T
    return out
